# revision 25
# baseline (speedup 1.0000x reference)
"""Tensor-parallel (head-sharded) Llama-style attention layer for 8 NeuronCores.

Problem shapes (hardcoded): B=2, S=2048, D=4096, NH=32 q-heads, NKV=8 kv-heads,
HD=128, causal prefill (input_pos == arange(S), mask == tril).

Sharding: core i gets q-heads 4i..4i+3 and kv-head i (wq/wk/wv output dims and
wo input dims sharded by head). x is replicated. Each core produces a partial
final output (its heads' contribution through wo); the host sums the 8 partials
(the "all-reduce after wo" done on host since the kernel returns full output).

Device kernel layout strategy: everything that feeds a matmul contraction is
kept [contraction-dim -> partitions]:
  phase 1: qT/kT = (wT chunk).T @ xT chunk  -> [head_dim, tokens], RoPE applied
           on a host-side de-interleaved head-dim permutation (pairs become
           halves, so the rotate is two contiguous partition-range copies).
           v is produced via vT then PE-transposed to natural [tokens, head_dim].
           K/V matmuls lag the Q matmuls by LAG chunks so the next token
           block's Q matmuls never wait on this block's K/V PSUM drain.
  phase 2: per (batch, q-block, head): scores_T[tk, tq] = kT.T @ qT in pairs of
           k-tiles into a 2-bank PSUM tile; exp on ScalarE (fused 1/sqrt(HD)
           scale, one ACTIVATE per full pair); causal masking by 0/1 mask
           multiply on the diagonal tiles (DVE); softmax denominator built by
           DVE accumulation of the exp tiles (esum) + one ones-matmul per
           head (replaces a ones-matmul per k-tile); reciprocal via the fast
           approx custom-DVE op; broadcast via a K=1 matmul; y normalization
           fused as one DVE multiply of two PSUM operands.
  phase 3: out[t, o] partial = yT_chunk.T @ woT chunk. The per-output-tile
           matmul groups are interleaved into the NEXT q-block's attention
           stream as PE fillers, so the PE never idles waiting on ScalarE's
           exp and the HAM clock gate keeps the PE at full frequency.

All matmul inputs use dtype float32r (full-rate fp32 on the PE at free-dim
>= 256; ~2^-13 mantissa rounding on read, far tighter than bf16).
"""

import math
from collections import deque
from contextlib import ExitStack

import ml_dtypes
import numpy as np

BF16 = ml_dtypes.bfloat16

B, S, D = 2, 2048, 4096
NH, NKV, HD = 32, 8, 128
NCORES = 8
QH = NH // NCORES  # q heads per core
EQ = QH * HD  # 512 = per-core q/o head-dim width
T = B * S  # 4096 total tokens
TB = 512  # token block (phase 1 / q blocks)
NTB = T // TB  # 8
DCH = D // 128  # 32 contraction chunks over model dim
NKT = S // 128  # 16 k tiles per batch
SCALE = 1.0 / math.sqrt(HD)
LAG = 8  # phase-1 K/V matmul lag (in chunks) behind Q

_NC_CACHE = {}


def _emit_phase1(nc, tc, ph1, mybir, tens, scratch, shared):
    """QKV projections + RoPE + v transpose."""
    F32 = mybir.dt.float32
    F32R = mybir.dt.float32r
    xT, wqT, wkT, wvT = tens["xT"], tens["wqT"], tens["wkT"], tens["wvT"]
    cosd, sind = tens["cosT"], tens["sinT"]
    ident = tens["ident"]
    identb = tens["identb"]
    qTs, kTs, vs = scratch["qTs"], scratch["kTs"], scratch["vs"]

    wpool = ph1.enter_context(tc.tile_pool(name="w1", bufs=1))
    # one big SBUF tile per weight, loaded by a few multi-chunk batched DMAs
    # (descriptor issue rate, not bandwidth, starves the PE with per-chunk
    # DMAs); they ride the Act HWDGE queue, off the x-tile stream
    BF16 = mybir.dt.bfloat16
    wq_all = wpool.tile([128, DCH * EQ], BF16, name="wq_all")
    wk_all = wpool.tile([128, DCH * HD], BF16, name="wk_all")
    wv_all = wpool.tile([128, DCH * HD], BF16, name="wv_all")
    WGB = 4  # chunks per wq DMA
    for g in range(DCH // WGB):
        nc.scalar.dma_start(
            wq_all[:, g * WGB * EQ : (g + 1) * WGB * EQ].rearrange(
                "p (c e) -> p c e", c=WGB
            ),
            wqT[g * WGB * 128 : (g + 1) * WGB * 128, :].rearrange(
                "(c p) e -> p c e", p=128
            ),
        )
    for g in range(2):
        h = DCH // 2
        nc.scalar.dma_start(
            wk_all[:, g * h * HD : (g + 1) * h * HD].rearrange(
                "p (c e) -> p c e", c=h
            ),
            wkT[g * h * 128 : (g + 1) * h * 128, :].rearrange(
                "(c p) e -> p c e", p=128
            ),
        )
        nc.scalar.dma_start(
            wv_all[:, g * h * HD : (g + 1) * h * HD].rearrange(
                "p (c e) -> p c e", c=h
            ),
            wvT[g * h * 128 : (g + 1) * h * 128, :].rearrange(
                "(c p) e -> p c e", p=128
            ),
        )
    wq_c = [wq_all[:, c * EQ : (c + 1) * EQ] for c in range(DCH)]
    wk_c = [wk_all[:, c * HD : (c + 1) * HD] for c in range(DCH)]
    wv_c = [wv_all[:, c * HD : (c + 1) * HD] for c in range(DCH)]

    xp = ph1.enter_context(tc.tile_pool(name="xp", bufs=8))
    rp = ph1.enter_context(tc.tile_pool(name="rope", bufs=2))
    sp1 = ph1.enter_context(tc.tile_pool(name="sp1", bufs=2))
    svp = ph1.enter_context(tc.tile_pool(name="svp", bufs=4))
    pp1 = ph1.enter_context(tc.tile_pool(name="pp1", bufs=1, space="PSUM"))
    pt1 = ph1.enter_context(tc.tile_pool(name="pt1", bufs=2, space="PSUM"))

    # warmup matmuls: keep the PE busy from t~0 so the HAM clock gate is
    # fully ramped when the first real matmul's inputs land
    for _ in range(28):
        wtile = pt1.tile([128, 128], F32, tag="vtr", name="warm")
        nc.tensor.matmul(wtile, ident, ident, start=True, stop=True)

    pending_vtr = [None]  # (sv_tile, b, ts0) from the previous token block

    def emit_vtr():
        if pending_vtr[0] is None:
            return
        sv, vb, vts0 = pending_vtr[0]
        pending_vtr[0] = None
        for u in range(TB // 128):
            ptr = pt1.tile([128, 128], F32, tag="vtr", name="ptr")
            nc.tensor.transpose(ptr, sv[:, u * 128 : (u + 1) * 128], ident)
            svT = svp.tile([128, 128], BF16, tag="svT", name="svT")
            nc.scalar.copy(svT, ptr)
            n = vts0 // 128 + u
            nc.sync.dma_start(vs[vb][:, n * HD : (n + 1) * HD], svT)

    for tb in range(NTB):
        t0 = tb * TB
        b = t0 // S
        ts0 = t0 % S

        psq = [
            pp1.tile([128, TB], F32, tag=f"q{j}", name=f"psq{j}")
            for j in range(QH)
        ]
        psk = pp1.tile([128, TB], F32, tag="k")
        psv = pp1.tile([128, TB], F32, tag="v")
        cos_blk = rp.tile([128, TB], F32, tag="cos")
        sin_blk = rp.tile([128, TB], F32, tag="sin")
        nc.sync.dma_start(cos_blk, cosd[:, ts0 : ts0 + TB])
        nc.sync.dma_start(sin_blk, sind[:, ts0 : ts0 + TB])
        xcs = {}
        for c in range(DCH):
            if c % 2 == 0:
                # one batched DMA covers two chunks (256 DRAM rows)
                xc2 = xp.tile([128, 2 * TB], BF16, tag="x")
                nc.sync.dma_start(
                    xc2[:, :].rearrange("p (c t) -> p c t", c=2),
                    xT[c * 128 : (c + 2) * 128, t0 : t0 + TB].rearrange(
                        "(c p) t -> p c t", p=128
                    ),
                )
                xcs[c] = xc2[:, 0:TB]
                xcs[c + 1] = xc2[:, TB : 2 * TB]
            xc = xcs[c]
            for j in range(QH):
                nc.tensor.matmul(
                    psq[j],
                    wq_c[c][:, j * 128 : (j + 1) * 128],
                    xc,
                    start=(c == 0),
                    stop=(c == DCH - 1),
                )
            if c == 1:
                # previous block's v transposes ride here: sv is long ready,
                # so the PE never waits on the post-tail psv drain
                emit_vtr()
                if tb == NTB // 2:
                    # b=0 k/v scratch fully stored by now: prefetch phase 2's
                    # SBUF copies (and the first q-block's q tiles) so the
                    # phase transition never waits on DMA
                    nc.sync.dma_start(shared["kt"][0], kTs[0])
                    nc.sync.dma_start(shared["vt"][0], vs[0])
                    for j in range(QH):
                        nc.sync.dma_start(
                            shared["qb0"][j], qTs[0][j * 128 : (j + 1) * 128, 0:TB]
                        )
                if tb == NTB - 2:
                    # wo + cmask prefetch: issued here (Act queue has slack)
                    # so their ring-credit stalls never delay phase 2's exps
                    nc.scalar.dma_start(shared["cmask"], tens["cmask"])
                    for j in range(QH):
                        nc.scalar.dma_start(
                            shared["wo"][j], tens["woT"][j * 128 : (j + 1) * 128, :]
                        )
            if c >= LAG:
                cc = c - LAG
                nc.tensor.matmul(
                    psk, wk_c[cc], xcs[cc], start=(cc == 0), stop=False
                )
                nc.tensor.matmul(
                    psv, wv_c[cc], xcs[cc], start=(cc == 0), stop=False
                )

        # drain the q PSUM tiles first (Act/DVE pairs run concurrently) so the
        # next block's q matmuls never wait; PE meanwhile runs the k/v tail
        srcs = {}
        for i, j in enumerate(range(QH)):
            src = sp1.tile([128, TB], F32, tag=f"srcq{j}", name="src")
            if i % 2 == 0:
                nc.scalar.copy(src, psq[j])
            else:
                nc.vector.tensor_copy(src, psq[j])
            srcs[j] = src

        for cc in range(DCH - LAG, DCH):
            last = cc == DCH - 1
            nc.tensor.matmul(psk, wk_c[cc], xcs[cc], start=False, stop=last)
            nc.tensor.matmul(psv, wv_c[cc], xcs[cc], start=False, stop=last)

        sv = sp1.tile([128, TB], F32, tag="sv")
        nc.scalar.copy(sv, psv)
        srck = sp1.tile([128, TB], F32, tag="srck", name="srck")
        nc.vector.tensor_copy(srck, psk)

        def rope_math(src):
            rot = rp.tile([128, TB], F32, tag="rot", name="rot")
            nc.vector.tensor_copy(rot[0:64, :], src[64:128, :])
            nc.vector.tensor_copy(rot[64:128, :], src[0:64, :])
            t1 = rp.tile([128, TB], F32, tag="t1", name="t1")
            nc.vector.tensor_mul(t1, src, cos_blk)
            nc.vector.tensor_mul(rot, rot, sin_blk)
            o = rp.tile([128, TB], BF16, tag="ro", name="ro")
            nc.vector.tensor_add(o, t1, rot)
            return o

        for j in range(QH):
            qr = rope_math(srcs[j])
            nc.sync.dma_start(qTs[b][j * 128 : (j + 1) * 128, ts0 : ts0 + TB], qr)
        kr = rope_math(srck)
        nc.sync.dma_start(kTs[b][:, ts0 : ts0 + TB], kr)

        pending_vtr[0] = (sv, b, ts0)
    emit_vtr()


def _emit_phase23(nc, tc, ph2, mybir, tens, scratch, shared):
    """Fused attention + output projection with PE-filler interleaving."""
    F32 = mybir.dt.float32
    F32R = mybir.dt.float32r
    Exp = mybir.ActivationFunctionType.Exp
    ones_t = tens["ones_t"]
    qTs, kTs, vs = scratch["qTs"], scratch["kTs"], scratch["vs"]
    woT, out = tens["woT"], tens["out"]

    BF16 = mybir.dt.bfloat16
    kv1 = ph2.enter_context(tc.tile_pool(name="kv1", bufs=1))
    shared["kt"][1] = kv1.tile([128, S], BF16, tag="kt1", name="kt_b1")
    shared["vt"][1] = kv1.tile([128, NKT * HD], BF16, tag="vt1", name="vt_b1")
    qp = ph2.enter_context(tc.tile_pool(name="qp", bufs=4))
    ep = ph2.enter_context(tc.tile_pool(name="ep", bufs=3))
    esp = ph2.enter_context(tc.tile_pool(name="esp", bufs=2))
    rcp = ph2.enter_context(tc.tile_pool(name="rcp", bufs=2))
    yp = ph2.enter_context(tc.tile_pool(name="yp", bufs=2))
    op = ph2.enter_context(tc.tile_pool(name="op", bufs=6))
    pps = ph2.enter_context(tc.tile_pool(name="pps", bufs=2, space="PSUM"))
    ppy = ph2.enter_context(tc.tile_pool(name="ppy", bufs=2, space="PSUM"))
    ppo = ph2.enter_context(tc.tile_pool(name="ppo", bufs=2, space="PSUM"))

    cmask_sb = shared["cmask"]
    wo_c = shared["wo"]

    octr = [0]
    pending_proj = deque()

    def emit_proj_tile(y_t, b, qb, u, ob):
        tt0 = b * S + qb * TB + u * 128
        p_o = ppo.tile([128, TB], F32, tag="po", name="p_o")
        for c in range(QH):
            nc.tensor.matmul(
                p_o,
                y_t[:, c, u * 128 : (u + 1) * 128],
                wo_c[c][:, ob * TB : (ob + 1) * TB],
                start=(c == 0),
                stop=(c == QH - 1),
            )
        o_sb = op.tile([128, TB], mybir.dt.bfloat16, tag="osb", name="o_sb")
        if octr[0] % 2 == 0:
            nc.vector.tensor_copy(o_sb, p_o)
        else:
            nc.scalar.copy(o_sb, p_o)
        octr[0] += 1
        nc.sync.dma_start(out[tt0 : tt0 + 128, ob * TB : (ob + 1) * TB], o_sb)

    norm_state = [None]  # pending per-head normalization: dict

    def flush_rs(st):
        p_rs = ppo.tile([128, TB], F32, tag="po", name="p_rs")
        nc.tensor.matmul(
            p_rs[0:1, :], ones_t[:, 0:1], st["esum"], start=True, stop=True
        )
        rec = rcp.tile([1, TB], F32, tag="rec", name="rec")
        with nc.allow_low_precision(reason="approx reciprocal, softmax denom"):
            nc.vector.reciprocal_approx_fast(rec, p_rs[0:1, :])
        recr = rcp.tile([1, TB], F32R, tag="recr", name="recr")
        nc.vector.tensor_copy(recr, rec)
        st["rec"] = recr

    def flush_bc(st):
        p_bc = ppo.tile([128, TB], F32, tag="po", name="p_bc")
        nc.tensor.matmul(
            p_bc,
            ones_t[0:1, :],
            st["rec"],
            start=True,
            stop=True,
        )
        bc_sb = rcp.tile([128, TB], F32, tag="bc", name="bc_sb")
        nc.scalar.copy(bc_sb, p_bc)
        nc.vector.tensor_mul(st["y_t"][:, st["h"], :], st["p_y"], bc_sb)

    def try_filler():
        # a filler is safe unless its y tile is the one the pending (not yet
        # emitted) normalization still has to write
        for _ in range(2):
            if pending_proj:
                st = norm_state[0]
                if st is None or st["y_t"] is not pending_proj[0][0]:
                    emit_proj_tile(*pending_proj.popleft())

    for b in range(B):
        kt_b = shared["kt"][b]
        vt_b = shared["vt"][b]
        for qb in range(S // TB):
            if b == 0 and qb == 1:
                # b=1's k/v SBUF loads: issued only now so they queue behind
                # the first q-block's loads, not ahead of them
                nc.sync.dma_start(shared["kt"][1], kTs[1])
                nc.sync.dma_start(shared["vt"][1], vs[1])
            y_t = yp.tile([128, QH, TB], mybir.dt.bfloat16, tag="yt", name="y_t")
            for h in range(QH):
                if b == 0 and qb == 0:
                    qblk = shared["qb0"][h]
                else:
                    qblk = qp.tile([128, TB], BF16, tag="qb", name="qblk")
                    nc.sync.dma_start(
                        qblk,
                        qTs[b][h * 128 : (h + 1) * 128, qb * TB : (qb + 1) * TB],
                    )
                nkt = (qb + 1) * (TB // 128)
                tiles = []
                for kt in range(nkt):
                    dj = kt - qb * (TB // 128)
                    if dj <= 0:
                        c0, mr = 0, None
                    elif dj == 1:
                        c0, mr = 128, (128, 256)
                    elif dj == 2:
                        c0, mr = 256, (256, 384)
                    else:
                        c0, mr = 384, (384, 512)
                    if dj == 0:
                        mr = (0, 128)
                    tiles.append((kt, dj, c0, mr))
                pairs = [tiles[i : i + 2] for i in range(0, nkt, 2)]

                p_y = ppy.tile([128, TB], F32, tag="py", name="p_y")
                esum = esp.tile([128, TB], F32R, tag="es", name="esum")

                def emit_pv(pr, e_t):
                    for sl, (kt, dj, c0, mr) in enumerate(pr):
                        nc.tensor.matmul(
                            p_y[:, c0:],
                            vt_b[:, kt * HD : (kt + 1) * HD],
                            e_t[:, sl * TB + c0 : (sl + 1) * TB],
                            start=(kt == 0),
                            stop=(kt == nkt - 1),
                            skip_group_check=True,
                        )

                prev = None
                for gi, pr in enumerate(pairs):
                    p_s = pps.tile([128, 2 * TB], F32, tag="ps", name="p_s")
                    for sl, (kt, dj, c0, mr) in enumerate(pr):
                        if b == 0 and qb == 0 and h == 0:
                            c0 = 0  # init the PSUM buffer full-width once
                        nc.tensor.matmul(
                            p_s[:, sl * TB + c0 : (sl + 1) * TB],
                            kt_b[:, kt * 128 : (kt + 1) * 128],
                            qblk[:, c0:],
                            start=True,
                            stop=True,
                        )
                    e_t = ep.tile([128, 2 * TB], BF16, tag="et", name="e_t")
                    # one ACTIVATE per pair: the unwritten hole in a diagonal
                    # pair's second slot holds old-but-finite scores (the
                    # first q-block writes both buffers full-width), and
                    # nothing ever reads the hole's exp values
                    s0 = pr[0][2]
                    nc.scalar.activation(
                        e_t[:, s0:], p_s[:, s0:], Exp, scale=SCALE
                    )
                    for sl, (kt, dj, c0, mr) in enumerate(pr):
                        if mr is not None:
                            m0, m1 = mr
                            nc.vector.tensor_mul(
                                e_t[:, sl * TB + m0 : sl * TB + m1],
                                e_t[:, sl * TB + m0 : sl * TB + m1],
                                cmask_sb[:, dj * TB + m0 : dj * TB + m1],
                            )
                    for sl, (kt, dj, c0, mr) in enumerate(pr):
                        if kt == 0:
                            nc.vector.tensor_copy(esum, e_t[:, 0:TB])
                        else:
                            nc.vector.tensor_add(
                                esum[:, c0:],
                                esum[:, c0:],
                                e_t[:, sl * TB + c0 : (sl + 1) * TB],
                            )
                    if gi == 1 and norm_state[0] is not None:
                        flush_rs(norm_state[0])
                    if prev is not None:
                        emit_pv(*prev)
                        try_filler()
                    if gi == 2 and norm_state[0] is not None:
                        flush_bc(norm_state[0])
                        norm_state[0] = None
                    prev = (pr, e_t)
                emit_pv(*prev)
                if norm_state[0] is not None:
                    # short heads (2 pairs): finish the previous norm now
                    flush_bc(norm_state[0])
                    norm_state[0] = None
                norm_state[0] = dict(
                    esum=esum, p_y=p_y, y_t=y_t, h=h, rec=None
                )
            # drain leftover fillers (all from the previous q-block), then
            # enqueue this q-block's projection tiles
            while pending_proj:
                emit_proj_tile(*pending_proj.popleft())
            for u in range(TB // 128):
                for ob in range(D // TB):
                    pending_proj.append((y_t, b, qb, u, ob))

    # final head's norm, then drain the last projection tiles
    if norm_state[0] is not None:
        flush_rs(norm_state[0])
        flush_bc(norm_state[0])
        norm_state[0] = None
    while pending_proj:
        emit_proj_tile(*pending_proj.popleft())


def _build_nc(phases=(1, 2, 3)):
    import concourse.bass as bass  # noqa: F401
    import concourse.tile as tile
    from concourse import bacc, mybir
    from concourse.masks import make_identity

    F32 = mybir.dt.float32
    F32R = mybir.dt.float32r

    nc = bacc.Bacc("TRN2", target_bir_lowering=False, debug=False, num_devices=NCORES)

    tens = {}
    BF16 = mybir.dt.bfloat16
    tens["xT"] = nc.dram_tensor("xT", [D, T], BF16, kind="ExternalInput").ap()
    tens["wqT"] = nc.dram_tensor("wqT", [D, EQ], BF16, kind="ExternalInput").ap()
    tens["wkT"] = nc.dram_tensor("wkT", [D, HD], BF16, kind="ExternalInput").ap()
    tens["wvT"] = nc.dram_tensor("wvT", [D, HD], BF16, kind="ExternalInput").ap()
    tens["woT"] = nc.dram_tensor("woT", [EQ, D], BF16, kind="ExternalInput").ap()
    tens["cosT"] = nc.dram_tensor("cosT", [HD, S], F32, kind="ExternalInput").ap()
    tens["sinT"] = nc.dram_tensor("sinT", [HD, S], F32, kind="ExternalInput").ap()
    tens["cmask"] = nc.dram_tensor(
        "cmask", [128, 4 * TB], BF16, kind="ExternalInput"
    ).ap()
    tens["identr"] = nc.dram_tensor(
        "identr", [128, 128], F32R, kind="ExternalInput"
    ).ap()
    tens["ones"] = nc.dram_tensor("ones", [128, 128], F32R, kind="ExternalInput").ap()
    tens["out"] = nc.dram_tensor("out", [T, D], BF16, kind="ExternalOutput").ap()

    with tile.TileContext(nc) as tc, ExitStack() as top:
        dram = top.enter_context(tc.tile_pool(name="dram", bufs=1, space="DRAM"))
        BF = mybir.dt.bfloat16
        scratch = {
            "qTs": [
                dram.tile([EQ, S], BF, name="qTs0"),
                dram.tile([EQ, S], BF, name="qTs1"),
            ],
            "kTs": [
                dram.tile([HD, S], BF, name="kTs0"),
                dram.tile([HD, S], BF, name="kTs1"),
            ],
            "vs": [
                dram.tile([128, NKT * HD], BF, name="vs0"),
                dram.tile([128, NKT * HD], BF, name="vs1"),
            ],
        }

        kvpool = top.enter_context(tc.tile_pool(name="kvs", bufs=1))
        shared = {
            # b=1's k/v tiles live in a phase-2-local pool (allocated after
            # the phase-1 pools are freed); only b=0's are prefetched in ph1
            "kt": [kvpool.tile([128, S], BF, tag="kt0", name="kt_b0"), None],
            "vt": [
                kvpool.tile([128, NKT * HD], BF, tag="vt0", name="vt_b0"),
                None,
            ],
            "qb0": [
                kvpool.tile([128, TB], BF, tag=f"qb0h{j}", name=f"qb0h{j}")
                for j in range(QH)
            ],
            "wo": [
                kvpool.tile([128, D], mybir.dt.bfloat16, tag=f"wo{j}", name=f"wo_c{j}")
                for j in range(QH)
            ],
            "cmask": kvpool.tile([128, 4 * TB], BF, tag="cm", name="cmask_sb"),
        }

        consts = top.enter_context(tc.tile_pool(name="consts", bufs=1))
        ones_t = consts.tile([128, 128], F32R)
        nc.sync.dma_start(ones_t, tens["ones"])
        tens["ones_t"] = ones_t
        ident = consts.tile([128, 128], F32)
        make_identity(nc, ident)
        tens["ident"] = ident
        identb = consts.tile([128, 128], mybir.dt.bfloat16)
        nc.vector.tensor_copy(identb, ident)
        tens["identb"] = identb

        if 1 in phases:
            with ExitStack() as ph1:
                _emit_phase1(nc, tc, ph1, mybir, tens, scratch, shared)

        if 2 in phases:
            with ExitStack() as ph2:
                _emit_phase23(nc, tc, ph2, mybir, tens, scratch, shared)

    nc.compile()
    return nc


def _get_nc():
    if "nc" not in _NC_CACHE:
        _NC_CACHE["nc"] = _build_nc()
    return _NC_CACHE["nc"]


def _host_prep(x, freqs_cos, freqs_sin, wq, wk, wv, wo):
    """Build per-core input maps (numpy only)."""
    x2d = np.ascontiguousarray(x.reshape(T, D).T)  # [D, T]
    x2d_bf = x2d.astype(BF16)

    # de-interleave permutation within each head: [r0..r63, i0..i63]
    perm = np.concatenate([np.arange(0, HD, 2), np.arange(1, HD, 2)])

    wq_h = wq.reshape(NH, HD, D)[:, perm, :].reshape(NH * HD, D)
    wk_h = wk.reshape(NKV, HD, D)[:, perm, :].reshape(NKV * HD, D)

    cos_de = np.empty((HD, S), np.float32)
    sin_de = np.empty((HD, S), np.float32)
    ft = freqs_cos.T  # [HD/2, S]
    st = freqs_sin.T
    cos_de[0:64] = ft
    cos_de[64:128] = ft
    sin_de[0:64] = -st
    sin_de[64:128] = st

    cmask = np.zeros((128, 4 * TB), np.float32)
    p = np.arange(128)[:, None]
    f = np.arange(TB)[None, :]
    for j in range(4):
        cmask[:, j * TB : (j + 1) * TB] = (p <= f - 128 * j).astype(np.float32)

    ones = np.ones((128, 128), np.float32)
    identr = np.eye(128, dtype=np.float32)

    in_maps = []
    for i in range(NCORES):
        qs = slice(i * EQ, (i + 1) * EQ)
        ks = slice(i * HD, (i + 1) * HD)
        in_maps.append(
            dict(
                xT=x2d_bf,
                wqT=np.ascontiguousarray(wq_h[qs].T).astype(BF16),
                wkT=np.ascontiguousarray(wk_h[ks].T).astype(BF16),
                wvT=np.ascontiguousarray(wv[ks].T).astype(BF16),
                woT=np.ascontiguousarray(wo[:, qs].T).astype(BF16),
                cosT=cos_de,
                sinT=sin_de,
                cmask=cmask.astype(BF16),
                ones=ones,
                identr=identr,
            )
        )
    return in_maps


def _numpy_fallback(x, freqs_cos, freqs_sin, wq, wk, wv, wo, k_cache, v_cache,
                    input_pos, mask):
    """Exact port of the reference for unexpected inputs. Slow but correct."""
    NREP = NH // NKV
    q = (x.reshape(T, D) @ wq.T).reshape(B, S, NH, HD)
    k = (x.reshape(T, D) @ wk.T).reshape(B, S, NKV, HD)
    v = (x.reshape(T, D) @ wv.T).reshape(B, S, NKV, HD)

    def rot(t):
        tr = t.reshape(*t.shape[:-1], HD // 2, 2)
        t_r, t_i = tr[..., 0], tr[..., 1]
        c = freqs_cos[None, :, None, :]
        s = freqs_sin[None, :, None, :]
        o_r = t_r * c - t_i * s
        o_i = t_r * s + t_i * c
        return np.stack([o_r, o_i], axis=-1).reshape(t.shape)

    q = rot(q).transpose(0, 2, 1, 3)
    k = rot(k).transpose(0, 2, 1, 3)
    v = v.transpose(0, 2, 1, 3)
    k_full = np.array(k_cache)
    v_full = np.array(v_cache)
    k_full[:, :, input_pos] = k
    v_full[:, :, input_pos] = v
    k_rep = np.repeat(k_full, NREP, axis=1)
    v_rep = np.repeat(v_full, NREP, axis=1)
    am = mask[input_pos][None, None]
    scores = np.einsum("bhqd,bhkd->bhqk", q, k_rep, optimize=True) * SCALE
    scores = np.where(am, scores, -np.inf)
    scores -= scores.max(axis=-1, keepdims=True)
    e = np.exp(scores)
    probs = e / e.sum(axis=-1, keepdims=True)
    y = np.einsum("bhqk,bhkd->bhqd", probs, v_rep, optimize=True)
    y = y.transpose(0, 2, 1, 3).reshape(B, S, NH * HD)
    return (y @ wo.T).astype(np.float32)


def kernel(**inputs):
    x = np.asarray(inputs["x"], np.float32)
    freqs_cos = np.asarray(inputs["freqs_cos"], np.float32)
    freqs_sin = np.asarray(inputs["freqs_sin"], np.float32)
    wq = np.asarray(inputs["wq"], np.float32)
    wk = np.asarray(inputs["wk"], np.float32)
    wv = np.asarray(inputs["wv"], np.float32)
    wo = np.asarray(inputs["wo"], np.float32)
    input_pos = np.asarray(inputs["input_pos"])
    mask = np.asarray(inputs["mask"])

    std = (
        np.array_equal(input_pos, np.arange(S, dtype=input_pos.dtype))
        and bool((mask == np.tril(np.ones((S, S), bool))).all())
    )
    if not std:
        return _numpy_fallback(
            x, freqs_cos, freqs_sin, wq, wk, wv, wo,
            inputs["k_cache"], inputs["v_cache"], input_pos, mask,
        )

    from concourse.bass_utils import run_bass_kernel_spmd

    nc = _get_nc()
    in_maps = _host_prep(x, freqs_cos, freqs_sin, wq, wk, wv, wo)
    res = run_bass_kernel_spmd(nc, in_maps, core_ids=list(range(NCORES)))
    acc = res.results[0]["out"].astype(np.float32)
    for r in res.results[1:]:
        acc = acc + r["out"]
    return acc.reshape(B, S, D).astype(np.float32)
